# revision 39
# baseline (speedup 1.0000x reference)
"""GQA attention kernel for 8 TRN2 NeuronCores (Bass/Tile) — v2.

Problem: h[2,2048,1024] -> out[2,2048,1024]
  q = h @ wq_w.T + wq_b   (16 heads x 64)
  k/v = h @ w{k,v}_w.T + b (4 KV groups x 64, each serves 4 consecutive heads)
  out = softmax(q k^T / 8) v

Sharding: 8 cores = 2 batches x 4 KV groups; each core does 4 query heads
against one K/V group. No collectives. Host pre-transposes / bf16-casts
inputs (layout prep) and re-assembles the output.

v2 vs baseline (baseline: 228us device measured / 248us TimelineSim;
v2: 164.5us TimelineSim, rel_err 8.8e-3 on HW; paired R=8 A/B measured
~188us/body faster than baseline on HW):
  - all matmul operands bf16 (halves h DMA + SBUF, kills f32r-rate risk)
  - kv-merged projection stationary (3 matmuls per (d,sq) instead of 4);
    kvT tile holds K on partitions 0-63 and V on 64-127; V transposed out
    via tile_position=(64,0) PE transposes interleaved with projection;
    half 1 projects sq-OUTER (3 psum tiles/pass) so pass 0's bias-adds +
    V-transposes overlap pass 1 and the proj-pool drain (which gates the
    attention PSUM pools opening) shrinks to one pass's adds
  - batched rearranged DMAs; bulk h/out transfers on the gpsimd queue;
    h streams half-0 columns of every d-slice before any half-1 bytes so
    the projection is never starved behind bytes it needs later; first
    512 cols and the d=0 weight slices go alone for the earliest start
  - two-deep software pipeline over 128 (block,kc) units: PE order is
    S(i), AV(i-2), with exp(i-1) on ACT in between -> PE never waits for
    exp latency; at pool bufs=5 breaks the exp->AV slot-recycle cycle
  - exp split ACT/DVE: every 4th unit uses a one-instruction Schraudolph
    exp on DVE (tensor_scalar affine to int16 bits == bf16 exp, max err
    ~3%), issued right after its scores for extra lead; keeps ACT (the
    1038ns/exp pacing engine) under the PE roofline
  - per-block tails (PE transpose back to [q,dh], reciprocal+scale on
    DVE) deferred 3 units and emitted 2 chunks/unit so the 2-bank tp
    pool rotation never stalls PE; bf16 output, one mid-kernel + one
    final batched out DMA

PSUM: scores 2x[128,1024] (4 banks) + po accumulator 2x[65,512] (2,
independently recycled per bank) + tail tp 2x[128,65] (2) = 8 banks.

Rejected after paired HW A/B (see memory): A-stationary AV restructure
(kernel5.py) — sims at 136us but per-matmul LDWEIGHTS (unmodeled) makes
it ~210us/body slower on hardware.
"""

import sys

for p in ("/opt/pypackages", "/opt/trn_rl_repo"):
    if p not in sys.path:
        sys.path.insert(0, p)

from contextlib import ExitStack

import numpy as np
import ml_dtypes

import concourse.bass as bass
import concourse.mybir as mybir
import concourse.tile as tile
from concourse import bacc
from concourse.bass_utils import run_bass_kernel_spmd
from concourse.masks import make_identity

F32 = mybir.dt.float32
BF16 = mybir.dt.bfloat16
I16 = mybir.dt.int16
NPBF = ml_dtypes.bfloat16

D_MODEL = 1024
SEQ = 2048
NHL = 4             # heads per core (one KV group)
DH = 64
QDIM = NHL * DH     # 256
BS = 2
NG = 4
ND = D_MODEL // 128  # 8 d-chunks
NS = SEQ // 128      # 16 seq chunks
NU = 8 * NS          # pipeline units: 8 (qh,l) blocks x 16 key-chunks

# Schraudolph exp -> bf16 bits via int16: i16 = s*A16 + B16, i16<<16 = f32
_LN2 = float(np.log(2.0))
A16 = (2.0 ** 23 / _LN2) / 65536.0
B16 = (127.0 * 2 ** 23 - 365000.0) / 65536.0
DVE_EXP_EVERY = 4    # units with i % this == DVE_EXP_PHASE use DVE exp
DVE_EXP_PHASE = 2
TAIL_DELAY = 3       # units into the next block before a block's PE tail
WIDE_MM = False      # single 1024-col matmuls for S/AV (PSUM bank-pair out)
H_DMA_2Q = False     # alternate h slices across gpsimd + scalar DMA queues


def build_program(reps=1):
    nc = bacc.Bacc("TRN2", target_bir_lowering=False, debug=False)

    hT_d = nc.dram_tensor("hT", [D_MODEL, SEQ], BF16, kind="ExternalInput").ap()
    wqT_d = nc.dram_tensor("wqT", [D_MODEL, QDIM], BF16, kind="ExternalInput").ap()
    wkvT_d = nc.dram_tensor("wkvT", [D_MODEL, 128], BF16, kind="ExternalInput").ap()
    bq_d = nc.dram_tensor("bq", [QDIM, 1], F32, kind="ExternalInput").ap()
    bkv_d = nc.dram_tensor("bkv", [128, 1], F32, kind="ExternalInput").ap()
    out_d = nc.dram_tensor("out", [SEQ, QDIM], BF16, kind="ExternalOutput").ap()

    with tile.TileContext(nc) as tc, ExitStack() as ctx:
        sb = ctx.enter_context(tc.tile_pool(name="sb", bufs=1))

        hTa = sb.tile([128, ND * SEQ], BF16, tag="hTa", name="hTa")
        wqa = sb.tile([128, ND * QDIM], BF16, tag="wqa", name="wqa")
        wkva = sb.tile([128, ND * 128], BF16, tag="wkva", name="wkva")
        bq0 = sb.tile([128, 1], F32, tag="bq0", name="bq0")
        bq1 = sb.tile([128, 1], F32, tag="bq1", name="bq1")
        bkv = sb.tile([128, 1], F32, tag="bkv", name="bkv")
        qt0 = sb.tile([128, SEQ], BF16, tag="qt0", name="qt0")  # heads 0,1
        qt1 = sb.tile([128, SEQ], BF16, tag="qt1", name="qt1")  # heads 2,3
        qh1 = sb.tile([DH, SEQ], BF16, tag="qh1", name="qh1")
        qh3 = sb.tile([DH, SEQ], BF16, tag="qh3", name="qh3")
        kvT = sb.tile([128, SEQ], BF16, tag="kvT", name="kvT")  # K rows 0-63, V 64-127
        vv = [sb.tile([128, DH + 1], BF16, tag=f"vv{i}", name=f"vv{i}")
              for i in range(NS)]
        identb = sb.tile([128, 128], BF16, tag="identb", name="identb")
        out_all = sb.tile([128, NS * QDIM], BF16, tag="out_all", name="out_all")

        make_identity(nc, identb[:, :])
        for rep in range(reps):
            _emit_body(nc, tc, rep, locals())

    nc.compile()
    return nc


def _emit_body(nc, tc, rep, env):
    hT_d, wqT_d, wkvT_d, bq_d, bkv_d, out_d = (
        env["hT_d"], env["wqT_d"], env["wkvT_d"], env["bq_d"], env["bkv_d"],
        env["out_d"])
    hTa, wqa, wkva, bq0, bq1, bkv = (
        env["hTa"], env["wqa"], env["wkva"], env["bq0"], env["bq1"], env["bkv"])
    qt0, qt1, qh1, qh3, kvT, vv, identb, out_all = (
        env["qt0"], env["qt1"], env["qh1"], env["qh3"], env["kvT"], env["vv"],
        env["identb"], env["out_all"])

    # ---- input DMAs: weights on SP queue, h bulk on gpsimd queue ----
    wq_src = wqT_d.rearrange("(d p) c -> p d c", d=ND)
    wqa_v = wqa[:, :].rearrange("p (d c) -> p d c", d=ND)
    wkv_src = wkvT_d.rearrange("(d p) c -> p d c", d=ND)
    wkva_v = wkva[:, :].rearrange("p (d c) -> p d c", d=ND)
    # d=0 slices first so the first projection matmuls start sooner
    nc.sync.dma_start(wqa_v[:, 0:1, :], wq_src[:, 0:1, :])
    nc.sync.dma_start(wkva_v[:, 0:1, :], wkv_src[:, 0:1, :])
    nc.sync.dma_start(wqa_v[:, 1:ND, :], wq_src[:, 1:ND, :])
    nc.sync.dma_start(wkva_v[:, 1:ND, :], wkv_src[:, 1:ND, :])
    nc.sync.dma_start(bq0[:, :], bq_d[0:128, :])
    nc.sync.dma_start(bq1[:, :], bq_d[128:256, :])
    nc.sync.dma_start(bkv[:, :], bkv_d[:, :])

    h_src = hT_d.rearrange("(d p) c -> p d c", d=ND)
    # stream the half-0 columns (0:1024) of every d-slice first -- half 1's
    # bytes would otherwise clog the stream ahead of what the projection
    # needs; the very first 512 go alone so the first matmul starts ASAP
    nc.gpsimd.dma_start(
        hTa[:, 0:512].rearrange("p (d c) -> p d c", d=1),
        h_src[:, 0:1, 0:512])
    nc.gpsimd.dma_start(
        hTa[:, 512:1024].rearrange("p (d c) -> p d c", d=1),
        h_src[:, 0:1, 512:1024])
    for d in range(1, ND):
        nc.gpsimd.dma_start(
            hTa[:, d * SEQ:d * SEQ + 1024].rearrange("p (d c) -> p d c", d=1),
            h_src[:, d:d + 1, 0:1024])
    for d in range(ND):
        nc.gpsimd.dma_start(
            hTa[:, d * SEQ + 1024:(d + 1) * SEQ].rearrange("p (d c) -> p d c", d=1),
            h_src[:, d:d + 1, 1024:SEQ])

    # ---- projections ----
    # half 0 is DMA-chase-bound: sq-inner d-loop (6 psum tiles) consumes
    # each h slice as it lands. half 1 runs sq-OUTER (3 tiles per pass) so
    # pass 0's bias-adds and V-transposes overlap pass 1's matmuls and the
    # pool drain that gates the attention PSUM pools shrinks to one pass.
    def _emit_adds_vtrans(half, sq, pt3, ptv):
        n0 = half * 1024 + sq * 512
        nc.vector.tensor_scalar_add(kvT[:, n0:n0 + 512], pt3[2][:, :], bkv[:, :])
        for i in range(n0 // 128, n0 // 128 + 4):
            p = ptv.tile([128, DH], BF16, tag="ptv", name="ptv")
            nc.tensor.transpose(p[:, :], kvT[64:128, i * 128:(i + 1) * 128],
                                identb[64:128, 64:128], tile_position=(64, 0))
            nc.vector.tensor_copy(vv[i][:, 0:DH], p[:, :])
            nc.vector.memset(vv[i][:, DH:DH + 1], 1.0)
        nc.vector.tensor_scalar_add(qt0[:, n0:n0 + 512], pt3[0][:, :], bq0[:, :])
        nc.vector.tensor_scalar_add(qt1[:, n0:n0 + 512], pt3[1][:, :], bq1[:, :])

    def _mm3(pt3, d, n0):
        rhs = hTa[:, d * SEQ + n0: d * SEQ + n0 + 512]
        st = dict(start=(d == 0), stop=(d == ND - 1))
        nc.tensor.matmul(pt3[0][:, :], wqa[:, d * QDIM:d * QDIM + 128], rhs, **st)
        nc.tensor.matmul(pt3[1][:, :], wqa[:, d * QDIM + 128:d * QDIM + 256], rhs, **st)
        nc.tensor.matmul(pt3[2][:, :], wkva[:, d * 128:(d + 1) * 128], rhs, **st)

    with tc.tile_pool(name=f"pp{rep}", bufs=1, space="PSUM") as pp, \
         tc.tile_pool(name=f"ptv{rep}", bufs=2, space="PSUM") as ptv:
        def tiles3(sq):
            return [pp.tile([128, 512], F32, tag=f"pp{sq}{t}", name=f"pp{sq}{t}")
                    for t in range(3)]
        pt = {sq: tiles3(sq) for sq in range(2)}
        for d in range(ND):
            for sq in range(2):
                _mm3(pt[sq], d, sq * 512)
        for sq in range(2):
            _emit_adds_vtrans(0, sq, pt[sq], ptv)
        for sq in range(2):
            pt3 = tiles3(sq)
            for d in range(ND):
                _mm3(pt3, d, 1024 + sq * 512)
            _emit_adds_vtrans(1, sq, pt3, ptv)

    # shift heads 1,3 down to partitions 0-63 (SBUF->SBUF DMA, SP queue)
    nc.sync.dma_start(qh1[:, :], qt0[64:128, :])
    nc.sync.dma_start(qh3[:, :], qt1[64:128, :])

    # ---- attention: flat pipeline over units i = block*16 + kc ----
    # block order (qh, l): all 4 heads for q-half 0, then q-half 1
    blocks = [(qh, l) for qh in range(2) for l in range(NHL)]
    qviews = [qt0[0:DH, :], qh1[:, :], qt1[0:DH, :], qh3[:, :]]

    with tc.tile_pool(name=f"psc{rep}", bufs=2, space="PSUM") as psc, \
         tc.tile_pool(name=f"po{rep}", bufs=1, space="PSUM") as pop, \
         tc.tile_pool(name=f"ptp{rep}", bufs=2, space="PSUM") as ptp, \
         tc.tile_pool(name=f"at{rep}", bufs=5) as atp, \
         tc.tile_pool(name=f"ot{rep}", bufs=2) as otp, \
         tc.tile_pool(name=f"rcp{rep}", bufs=4) as rcp:

        po = {}
        ps_prev = at_prev = None
        tails = {}  # trigger unit -> block index

        def emit_scores(i):
            b, kc = divmod(i, NS)
            qh, l = blocks[b]
            ps = psc.tile([128, 1024], F32, tag="ps", name="ps")
            if WIDE_MM:
                nc.tensor.matmul(
                    ps[:, :], kvT[0:DH, kc * 128:(kc + 1) * 128],
                    qviews[l][:, qh * 1024: qh * 1024 + 1024],
                    start=True, stop=True)
            else:
                for n in range(2):
                    nc.tensor.matmul(
                        ps[:, n * 512:(n + 1) * 512],
                        kvT[0:DH, kc * 128:(kc + 1) * 128],
                        qviews[l][:, qh * 1024 + n * 512: qh * 1024 + (n + 1) * 512],
                        start=True, stop=True)
            return ps

        def emit_exp(i, ps):
            at = atp.tile([128, 1024], BF16, tag="at", name="at")
            if i % DVE_EXP_EVERY == DVE_EXP_PHASE:
                nc.vector.tensor_scalar(
                    at.bitcast(I16)[:, :], ps[:, :], A16, B16,
                    op0=mybir.AluOpType.mult, op1=mybir.AluOpType.add)
            else:
                nc.scalar.activation(at[:, :], ps[:, :],
                                     mybir.ActivationFunctionType.Exp)
            return at

        def emit_av(i, at):
            b, kc = divmod(i, NS)
            if kc == 0:
                po[b] = [pop.tile([DH + 1, 512], F32, tag=f"po{n}", name="po")
                         for n in range(2)]
            for n in range(2):
                nc.tensor.matmul(
                    po[b][n][:, :],
                    vv[kc][:, :], at[:, n * 512:(n + 1) * 512],
                    start=(kc == 0), stop=(kc == NS - 1))
            if kc == NS - 1:
                # copy to SBUF now (frees po for the next block); per-bank
                # tiles so each recycles as soon as its copy is done
                ot = otp.tile([DH + 1, 1024], BF16, tag="ot", name="ot")
                nc.vector.tensor_copy(ot[:, 0:512], po[b][0][:, :])
                nc.vector.tensor_copy(ot[:, 512:1024], po[b][1][:, :])
                delay = TAIL_DELAY if b < len(blocks) - 1 else 1
                tails.setdefault(i + 1 + delay, []).append((b, ot, 0))

        def emit_tail_pair(trigger, b, ot, j0):
            # two 128-q chunks per unit so the 2-slot tp rotation stays
            # ahead of DVE; reschedule the rest for the next unit
            qh, l = blocks[b]
            for j in (j0, j0 + 1):
                tp = ptp.tile([128, DH + 1], BF16, tag="tp", name="tp")
                nc.tensor.transpose(tp[:, :], ot[:, j * 128:(j + 1) * 128],
                                    identb[0:DH + 1, 0:DH + 1])
                rc = rcp.tile([128, 1], F32, tag="rc", name="rc")
                nc.vector.reciprocal(rc[:, :], tp[:, DH:DH + 1])
                qc = qh * 8 + j
                nc.vector.tensor_scalar_mul(
                    out_all[:, qc * QDIM + l * DH: qc * QDIM + (l + 1) * DH],
                    tp[:, 0:DH], rc[:, :])
            if b == len(blocks) - 1:
                # last block: these 2 chunks are now fully written by all
                # heads -- stream them out while remaining pairs finish
                nc.gpsimd.dma_start(
                    out_d.rearrange("(i p) c -> p i c", i=NS)[:, 8 + j0:10 + j0, :],
                    out_all[:, (8 + j0) * QDIM:(10 + j0) * QDIM].rearrange(
                        "p (i c) -> p i c", i=2))
            if j0 + 2 < 8:
                tails.setdefault(trigger + 1, []).append((b, ot, j0 + 2))

        # two-deep pipeline: PE order S(i), AV(i-2); exp(i-1) lands between
        # so PE never waits on ACT/DVE exp latency
        ats = {}
        for i in range(NU + 7):
            if i < NU:
                ps = emit_scores(i)
                if i % DVE_EXP_EVERY == DVE_EXP_PHASE:
                    # DVE exp issued immediately: extra lead for the slower
                    # (and queue-contended) DVE path
                    ats[i] = emit_exp(i, ps)
            if 2 <= i <= NU + 1:
                emit_av(i - 2, ats.pop(i - 2))
            if 1 <= i <= NU and i - 1 < NU and (i - 1) not in ats \
                    and (i - 1) % DVE_EXP_EVERY != DVE_EXP_PHASE:
                ats[i - 1] = emit_exp(i - 1, ps_prev)
            for (b, ot, j0) in tails.pop(i, []):
                emit_tail_pair(i, b, ot, j0)
            # first-half output DMA once blocks 0-3 (qh=0) tails are emitted
            if i == 4 * NS + TAIL_DELAY + 5:
                nc.gpsimd.dma_start(
                    out_d.rearrange("(i p) c -> p i c", i=NS)[:, 0:8, :],
                    out_all[:, 0:8 * QDIM].rearrange("p (i c) -> p i c", i=8))
            if i < NU:
                ps_prev = ps
        assert not tails and not ats


_NC = None
LAST_RESULTS = None
LAST_IN_MAPS = None


def kernel(h, wq_w, wq_b, wk_w, wk_b, wv_w, wv_b, **kw):
    global _NC, LAST_RESULTS, LAST_IN_MAPS
    if _NC is None:
        _NC = build_program()

    h = np.asarray(h, np.float32)
    wq_w = np.asarray(wq_w, np.float32)
    wq_b = np.asarray(wq_b, np.float32)
    wk_w = np.asarray(wk_w, np.float32)
    wk_b = np.asarray(wk_b, np.float32)
    wv_w = np.asarray(wv_w, np.float32)
    wv_b = np.asarray(wv_b, np.float32)

    in_maps = []
    for core in range(8):
        b, g = divmod(core, NG)
        # fold the 1/sqrt(dh) score scale into wq/bq
        wq_s = wq_w[g * QDIM:(g + 1) * QDIM, :] * 0.125
        bq_s = wq_b[g * QDIM:(g + 1) * QDIM] * 0.125
        wk_s = wk_w[g * DH:(g + 1) * DH, :]
        wv_s = wv_w[g * DH:(g + 1) * DH, :]
        wkv = np.concatenate([wk_s, wv_s], axis=0)          # [128, 1024]
        bkv = np.concatenate([wk_b[g * DH:(g + 1) * DH],
                              wv_b[g * DH:(g + 1) * DH]])   # [128]
        in_maps.append({
            "hT": np.ascontiguousarray(h[b].T).astype(NPBF),
            "wqT": np.ascontiguousarray(wq_s.T).astype(NPBF),
            "wkvT": np.ascontiguousarray(wkv.T).astype(NPBF),
            "bq": np.ascontiguousarray(bq_s.reshape(QDIM, 1)),
            "bkv": np.ascontiguousarray(bkv.reshape(128, 1)),
        })

    res = run_bass_kernel_spmd(_NC, in_maps, core_ids=list(range(8)))
    LAST_RESULTS = res
    LAST_IN_MAPS = in_maps

    out = np.empty((BS, SEQ, 1024), np.float32)
    for core in range(8):
        b, g = divmod(core, NG)
        out[b, :, g * QDIM:(g + 1) * QDIM] = res.results[core]["out"].astype(
            np.float32)
    return out


def bench_exec_ns(reps=8, iters=4):
    """Per-NEFF-execution time: stream `reps` async dispatches with
    device-resident inputs; subtract a reps=1 launch and divide."""
    import time

    import jax
    from jax.sharding import Mesh, NamedSharding, PartitionSpec
    from jax.experimental.shard_map import shard_map

    from concourse import bass2jax, mybir as _mb

    assert _NC is not None and LAST_IN_MAPS is not None, "call kernel() first"
    nc = _NC
    bass2jax.install_neuronx_cc_hook()
    partition_name = (nc.partition_id_tensor.name
                      if nc.partition_id_tensor else None)

    in_names, out_names, out_avals, zero_outs = [], [], [], []
    for alloc in nc.m.functions[0].allocations:
        if not isinstance(alloc, _mb.MemoryLocationSet):
            continue
        name = alloc.memorylocations[0].name
        if alloc.kind == "ExternalInput":
            if name != partition_name:
                in_names.append(name)
        elif alloc.kind == "ExternalOutput":
            out_names.append(name)
            shape = tuple(alloc.tensor_shape)
            dtype = _mb.dt.np(alloc.dtype)
            out_avals.append(jax.core.ShapedArray(shape, dtype))
            zero_outs.append(np.zeros(shape, dtype))
    n_params = len(in_names)
    all_in_names = in_names + out_names
    if partition_name is not None:
        all_in_names.append(partition_name)

    def _body(*args):
        ins = list(args[:n_params])
        outs = list(args[n_params:])
        pid = ([bass2jax.partition_id_tensor()]
               if partition_name is not None else [])
        outs = list(bass2jax._bass_exec_p.bind(
            *ins, *outs, *pid,
            out_avals=tuple(out_avals),
            in_names=tuple(all_in_names),
            out_names=tuple(out_names),
            lowering_input_output_aliases=(),
            sim_require_finite=True,
            sim_require_nnan=True,
            nc=nc,
        ))
        return tuple(outs)

    devices = jax.devices()[:8]
    mesh = Mesh(np.asarray(devices), ("core",))
    spec = PartitionSpec("core")
    n_outs = len(out_names)
    concat_in = [
        np.concatenate([np.asarray(m[name]) for m in LAST_IN_MAPS], axis=0)
        for name in in_names
    ]
    concat_zeros = [np.zeros((8 * z.shape[0], *z.shape[1:]), z.dtype)
                    for z in zero_outs]
    sh = NamedSharding(mesh, spec)
    dev_args = [jax.device_put(a, sh) for a in concat_in + concat_zeros]

    fn = jax.jit(shard_map(_body, mesh=mesh,
                           in_specs=(spec,) * (n_params + n_outs),
                           out_specs=(spec,) * n_outs, check_rep=False))
    r = fn(*dev_args)  # compile + warm
    jax.block_until_ready(r)

    times = {}
    for n in (1, reps):
        best = float("inf")
        for _ in range(iters):
            t0 = time.perf_counter()
            rs = [fn(*dev_args) for _ in range(n)]
            jax.block_until_ready(rs)
            best = min(best, time.perf_counter() - t0)
        times[n] = best
    per_exec = (times[reps] - times[1]) / (reps - 1)
    return per_exec * 1e9, times


# revision 41
# speedup vs baseline: 1.3129x; 1.3129x over previous
"""GQA attention kernel for 8 TRN2 NeuronCores (Bass/Tile) — v2.

Problem: h[2,2048,1024] -> out[2,2048,1024]
  q = h @ wq_w.T + wq_b   (16 heads x 64)
  k/v = h @ w{k,v}_w.T + b (4 KV groups x 64, each serves 4 consecutive heads)
  out = softmax(q k^T / 8) v

Sharding: 8 cores = 2 batches x 4 KV groups; each core does 4 query heads
against one K/V group. No collectives. Host pre-transposes / bf16-casts
inputs (layout prep) and re-assembles the output.

v2 vs baseline (baseline: 228us device measured / 248us TimelineSim;
v2: 164.5us TimelineSim, rel_err 8.8e-3 on HW; paired R=8 A/B measured
~188us/body faster than baseline on HW):
  - all matmul operands bf16 (halves h DMA + SBUF, kills f32r-rate risk)
  - kv-merged projection stationary (3 matmuls per (d,sq) instead of 4);
    kvT tile holds K on partitions 0-63 and V on 64-127; V transposed out
    via tile_position=(64,0) PE transposes interleaved with projection;
    half 1 projects sq-OUTER (3 psum tiles/pass) so pass 0's bias-adds +
    V-transposes overlap pass 1 and the proj-pool drain (which gates the
    attention PSUM pools opening) shrinks to one pass's adds
  - batched rearranged DMAs; bulk h/out transfers on the gpsimd queue;
    h streams half-0 columns of every d-slice before any half-1 bytes so
    the projection is never starved behind bytes it needs later; first
    512 cols and the d=0 weight slices go alone for the earliest start
  - two-deep software pipeline over 128 (block,kc) units: PE order is
    S(i), AV(i-2), with exp(i-1) on ACT in between -> PE never waits for
    exp latency; at pool bufs=5 breaks the exp->AV slot-recycle cycle
  - exp split ACT/DVE: every 4th unit uses a one-instruction Schraudolph
    exp on DVE (tensor_scalar affine to int16 bits == bf16 exp, max err
    ~3%), issued right after its scores for extra lead; keeps ACT (the
    1038ns/exp pacing engine) under the PE roofline
  - per-block tails (PE transpose back to [q,dh], reciprocal+scale on
    DVE) deferred 3 units and emitted 2 chunks/unit so the 2-bank tp
    pool rotation never stalls PE; bf16 output, one mid-kernel + one
    final batched out DMA

PSUM: scores 2x[128,1024] (4 banks) + po accumulator 2x[65,512] (2,
independently recycled per bank) + tail tp 2x[128,65] (2) = 8 banks.

Rejected after paired HW A/B (see memory): A-stationary AV restructure
(kernel5.py) — sims at 136us but per-matmul LDWEIGHTS (unmodeled) makes
it ~210us/body slower on hardware.
"""

import sys

for p in ("/opt/pypackages", "/opt/trn_rl_repo"):
    if p not in sys.path:
        sys.path.insert(0, p)

from contextlib import ExitStack

import numpy as np
import ml_dtypes

import concourse.bass as bass
import concourse.mybir as mybir
import concourse.tile as tile
from concourse import bacc
from concourse.bass_utils import run_bass_kernel_spmd
from concourse.masks import make_identity

F32 = mybir.dt.float32
BF16 = mybir.dt.bfloat16
I16 = mybir.dt.int16
NPBF = ml_dtypes.bfloat16

D_MODEL = 1024
SEQ = 2048
NHL = 4             # heads per core (one KV group)
DH = 64
QDIM = NHL * DH     # 256
BS = 2
NG = 4
ND = D_MODEL // 128  # 8 d-chunks
NS = SEQ // 128      # 16 seq chunks
NU = 8 * NS          # pipeline units: 8 (qh,l) blocks x 16 key-chunks

# Schraudolph exp -> bf16 bits via int16: i16 = s*A16 + B16, i16<<16 = f32
_LN2 = float(np.log(2.0))
A16 = (2.0 ** 23 / _LN2) / 65536.0
B16 = (127.0 * 2 ** 23 - 365000.0) / 65536.0
DVE_EXP_EVERY = 4    # units with i % this == DVE_EXP_PHASE use DVE exp
DVE_EXP_PHASE = 2
TAIL_DELAY = 3       # units into the next block before a block's PE tail
WIDE_MM = False      # single 1024-col matmuls for S/AV (PSUM bank-pair out)
H_DMA_2Q = False     # alternate h slices across gpsimd + scalar DMA queues


def build_program(reps=1):
    nc = bacc.Bacc("TRN2", target_bir_lowering=False, debug=False)

    hT_d = nc.dram_tensor("hT", [D_MODEL, SEQ], BF16, kind="ExternalInput").ap()
    wqT_d = nc.dram_tensor("wqT", [D_MODEL, QDIM], BF16, kind="ExternalInput").ap()
    wkvT_d = nc.dram_tensor("wkvT", [D_MODEL, 128], BF16, kind="ExternalInput").ap()
    bq_d = nc.dram_tensor("bq", [QDIM, 1], F32, kind="ExternalInput").ap()
    bkv_d = nc.dram_tensor("bkv", [128, 1], F32, kind="ExternalInput").ap()
    out_d = nc.dram_tensor("out", [SEQ, QDIM], BF16, kind="ExternalOutput").ap()

    with tile.TileContext(nc) as tc, ExitStack() as ctx:
        sb = ctx.enter_context(tc.tile_pool(name="sb", bufs=1))

        hTa = sb.tile([128, ND * SEQ], BF16, tag="hTa", name="hTa")
        wqa = sb.tile([128, ND * QDIM], BF16, tag="wqa", name="wqa")
        wkva = sb.tile([128, ND * 128], BF16, tag="wkva", name="wkva")
        bq0 = sb.tile([128, 1], F32, tag="bq0", name="bq0")
        bq1 = sb.tile([128, 1], F32, tag="bq1", name="bq1")
        bkv = sb.tile([128, 1], F32, tag="bkv", name="bkv")
        qt0 = sb.tile([128, SEQ], BF16, tag="qt0", name="qt0")  # heads 0,1
        qt1 = sb.tile([128, SEQ], BF16, tag="qt1", name="qt1")  # heads 2,3
        qh1 = sb.tile([DH, SEQ], BF16, tag="qh1", name="qh1")
        qh3 = sb.tile([DH, SEQ], BF16, tag="qh3", name="qh3")
        kvT = sb.tile([128, SEQ], BF16, tag="kvT", name="kvT")  # K rows 0-63, V 64-127
        vv = [sb.tile([128, DH + 1], BF16, tag=f"vv{i}", name=f"vv{i}")
              for i in range(NS)]
        identb = sb.tile([128, 128], BF16, tag="identb", name="identb")
        out_all = sb.tile([128, NS * QDIM], BF16, tag="out_all", name="out_all")

        make_identity(nc, identb[:, :])
        for rep in range(reps):
            _emit_body(nc, tc, rep, locals())

    nc.compile()
    return nc


def _emit_body(nc, tc, rep, env):
    hT_d, wqT_d, wkvT_d, bq_d, bkv_d, out_d = (
        env["hT_d"], env["wqT_d"], env["wkvT_d"], env["bq_d"], env["bkv_d"],
        env["out_d"])
    hTa, wqa, wkva, bq0, bq1, bkv = (
        env["hTa"], env["wqa"], env["wkva"], env["bq0"], env["bq1"], env["bkv"])
    qt0, qt1, qh1, qh3, kvT, vv, identb, out_all = (
        env["qt0"], env["qt1"], env["qh1"], env["qh3"], env["kvT"], env["vv"],
        env["identb"], env["out_all"])

    # ---- input DMAs: weights on SP queue, h bulk on gpsimd queue ----
    wq_src = wqT_d.rearrange("(d p) c -> p d c", d=ND)
    wqa_v = wqa[:, :].rearrange("p (d c) -> p d c", d=ND)
    wkv_src = wkvT_d.rearrange("(d p) c -> p d c", d=ND)
    wkva_v = wkva[:, :].rearrange("p (d c) -> p d c", d=ND)
    # d=0 slices first so the first projection matmuls start sooner
    nc.sync.dma_start(wqa_v[:, 0:1, :], wq_src[:, 0:1, :])
    nc.sync.dma_start(wkva_v[:, 0:1, :], wkv_src[:, 0:1, :])
    nc.sync.dma_start(wqa_v[:, 1:ND, :], wq_src[:, 1:ND, :])
    nc.sync.dma_start(wkva_v[:, 1:ND, :], wkv_src[:, 1:ND, :])
    nc.sync.dma_start(bq0[:, :], bq_d[0:128, :])
    nc.sync.dma_start(bq1[:, :], bq_d[128:256, :])
    nc.sync.dma_start(bkv[:, :], bkv_d[:, :])

    h_src = hT_d.rearrange("(d p) c -> p d c", d=ND)
    # stream the half-0 columns (0:1024) of every d-slice first -- half 1's
    # bytes would otherwise clog the stream ahead of what the projection
    # needs; the very first 512 go alone so the first matmul starts ASAP
    nc.gpsimd.dma_start(
        hTa[:, 0:512].rearrange("p (d c) -> p d c", d=1),
        h_src[:, 0:1, 0:512])
    nc.gpsimd.dma_start(
        hTa[:, 512:1024].rearrange("p (d c) -> p d c", d=1),
        h_src[:, 0:1, 512:1024])
    for d in range(1, ND):
        nc.gpsimd.dma_start(
            hTa[:, d * SEQ:d * SEQ + 1024].rearrange("p (d c) -> p d c", d=1),
            h_src[:, d:d + 1, 0:1024])
    for d in range(ND):
        nc.gpsimd.dma_start(
            hTa[:, d * SEQ + 1024:(d + 1) * SEQ].rearrange("p (d c) -> p d c", d=1),
            h_src[:, d:d + 1, 1024:SEQ])

    # ---- projections ----
    # half 0 is DMA-chase-bound: sq-inner d-loop (6 psum tiles) consumes
    # each h slice as it lands. half 1 runs sq-OUTER (3 tiles per pass) so
    # pass 0's bias-adds and V-transposes overlap pass 1's matmuls and the
    # pool drain that gates the attention PSUM pools shrinks to one pass.
    def _emit_adds_vtrans(half, sq, pt3, ptv):
        n0 = half * 1024 + sq * 512
        nc.vector.tensor_scalar_add(kvT[:, n0:n0 + 512], pt3[2][:, :], bkv[:, :])
        for i in range(n0 // 128, n0 // 128 + 4):
            p = ptv.tile([128, DH], BF16, tag="ptv", name="ptv")
            nc.tensor.transpose(p[:, :], kvT[64:128, i * 128:(i + 1) * 128],
                                identb[64:128, 64:128], tile_position=(64, 0))
            nc.vector.tensor_copy(vv[i][:, 0:DH], p[:, :])
            nc.vector.memset(vv[i][:, DH:DH + 1], 1.0)
        nc.vector.tensor_scalar_add(qt0[:, n0:n0 + 512], pt3[0][:, :], bq0[:, :])
        nc.vector.tensor_scalar_add(qt1[:, n0:n0 + 512], pt3[1][:, :], bq1[:, :])

    def _mm3(pt3, d, n0):
        rhs = hTa[:, d * SEQ + n0: d * SEQ + n0 + 512]
        st = dict(start=(d == 0), stop=(d == ND - 1))
        nc.tensor.matmul(pt3[0][:, :], wqa[:, d * QDIM:d * QDIM + 128], rhs, **st)
        nc.tensor.matmul(pt3[1][:, :], wqa[:, d * QDIM + 128:d * QDIM + 256], rhs, **st)
        nc.tensor.matmul(pt3[2][:, :], wkva[:, d * 128:(d + 1) * 128], rhs, **st)

    with tc.tile_pool(name=f"pp{rep}", bufs=1, space="PSUM") as pp, \
         tc.tile_pool(name=f"ptv{rep}", bufs=2, space="PSUM") as ptv:
        def tiles3(sq):
            return [pp.tile([128, 512], F32, tag=f"pp{sq}{t}", name=f"pp{sq}{t}")
                    for t in range(3)]
        pt = {sq: tiles3(sq) for sq in range(2)}
        for d in range(ND):
            for sq in range(2):
                _mm3(pt[sq], d, sq * 512)
        for sq in range(2):
            _emit_adds_vtrans(0, sq, pt[sq], ptv)
        for sq in range(2):
            pt3 = tiles3(sq)
            for d in range(ND):
                _mm3(pt3, d, 1024 + sq * 512)
            _emit_adds_vtrans(1, sq, pt3, ptv)

    # shift heads 1,3 down to partitions 0-63 (SBUF->SBUF DMA, SP queue)
    nc.sync.dma_start(qh1[:, :], qt0[64:128, :])
    nc.sync.dma_start(qh3[:, :], qt1[64:128, :])

    # ---- attention: flat pipeline over units i = block*16 + kc ----
    # block order (qh, l): all 4 heads for q-half 0, then q-half 1
    blocks = [(qh, l) for qh in range(2) for l in range(NHL)]
    qviews = [qt0[0:DH, :], qh1[:, :], qt1[0:DH, :], qh3[:, :]]

    with tc.tile_pool(name=f"psc{rep}", bufs=2, space="PSUM") as psc, \
         tc.tile_pool(name=f"po{rep}", bufs=1, space="PSUM") as pop, \
         tc.tile_pool(name=f"ptp{rep}", bufs=2, space="PSUM") as ptp, \
         tc.tile_pool(name=f"at{rep}", bufs=5) as atp, \
         tc.tile_pool(name=f"ot{rep}", bufs=2) as otp, \
         tc.tile_pool(name=f"rcp{rep}", bufs=4) as rcp:

        po = {}
        ps_prev = at_prev = None
        tails = {}  # trigger unit -> block index

        def emit_scores(i):
            b, kc = divmod(i, NS)
            qh, l = blocks[b]
            ps = psc.tile([128, 1024], F32, tag="ps", name="ps")
            if WIDE_MM:
                nc.tensor.matmul(
                    ps[:, :], kvT[0:DH, kc * 128:(kc + 1) * 128],
                    qviews[l][:, qh * 1024: qh * 1024 + 1024],
                    start=True, stop=True)
            else:
                for n in range(2):
                    nc.tensor.matmul(
                        ps[:, n * 512:(n + 1) * 512],
                        kvT[0:DH, kc * 128:(kc + 1) * 128],
                        qviews[l][:, qh * 1024 + n * 512: qh * 1024 + (n + 1) * 512],
                        start=True, stop=True)
            return ps

        def emit_exp(i, ps):
            at = atp.tile([128, 1024], BF16, tag="at", name="at")
            if i % DVE_EXP_EVERY == DVE_EXP_PHASE:
                nc.vector.tensor_scalar(
                    at.bitcast(I16)[:, :], ps[:, :], A16, B16,
                    op0=mybir.AluOpType.mult, op1=mybir.AluOpType.add)
            else:
                nc.scalar.activation(at[:, :], ps[:, :],
                                     mybir.ActivationFunctionType.Exp)
            return at

        def emit_av(i, at):
            b, kc = divmod(i, NS)
            if kc == 0:
                po[b] = [pop.tile([DH + 1, 512], F32, tag=f"po{n}", name="po")
                         for n in range(2)]
            for n in range(2):
                nc.tensor.matmul(
                    po[b][n][:, :],
                    vv[kc][:, :], at[:, n * 512:(n + 1) * 512],
                    start=(kc == 0), stop=(kc == NS - 1))
            if kc == NS - 1:
                # copy to SBUF now (frees po for the next block); per-bank
                # tiles so each recycles as soon as its copy is done
                ot = otp.tile([DH + 1, 1024], BF16, tag="ot", name="ot")
                nc.vector.tensor_copy(ot[:, 0:512], po[b][0][:, :])
                nc.vector.tensor_copy(ot[:, 512:1024], po[b][1][:, :])
                delay = TAIL_DELAY if b < len(blocks) - 1 else 1
                tails.setdefault(i + 1 + delay, []).append((b, ot, 0))

        def emit_tail_pair(trigger, b, ot, j0):
            # two 128-q chunks per unit so the 2-slot tp rotation stays
            # ahead of DVE; reschedule the rest for the next unit
            qh, l = blocks[b]
            for j in (j0, j0 + 1):
                tp = ptp.tile([128, DH + 1], BF16, tag="tp", name="tp")
                nc.tensor.transpose(tp[:, :], ot[:, j * 128:(j + 1) * 128],
                                    identb[0:DH + 1, 0:DH + 1])
                rc = rcp.tile([128, 1], F32, tag="rc", name="rc")
                nc.vector.reciprocal(rc[:, :], tp[:, DH:DH + 1])
                qc = qh * 8 + j
                nc.vector.tensor_scalar_mul(
                    out_all[:, qc * QDIM + l * DH: qc * QDIM + (l + 1) * DH],
                    tp[:, 0:DH], rc[:, :])
            if b == len(blocks) - 1:
                # last block: these 2 chunks are now fully written by all
                # heads -- stream them out while remaining pairs finish
                nc.gpsimd.dma_start(
                    out_d.rearrange("(i p) c -> p i c", i=NS)[:, 8 + j0:10 + j0, :],
                    out_all[:, (8 + j0) * QDIM:(10 + j0) * QDIM].rearrange(
                        "p (i c) -> p i c", i=2))
            if j0 + 2 < 8:
                tails.setdefault(trigger + 1, []).append((b, ot, j0 + 2))

        # two-deep pipeline: PE order S(i), AV(i-2); exp(i-1) lands between
        # so PE never waits on ACT/DVE exp latency
        ats = {}
        for i in range(NU + 7):
            if i < NU:
                ps = emit_scores(i)
                if i % DVE_EXP_EVERY == DVE_EXP_PHASE:
                    # DVE exp issued immediately: extra lead for the slower
                    # (and queue-contended) DVE path
                    ats[i] = emit_exp(i, ps)
            if 2 <= i <= NU + 1:
                emit_av(i - 2, ats.pop(i - 2))
            if 1 <= i <= NU and i - 1 < NU and (i - 1) not in ats \
                    and (i - 1) % DVE_EXP_EVERY != DVE_EXP_PHASE:
                ats[i - 1] = emit_exp(i - 1, ps_prev)
            for (b, ot, j0) in tails.pop(i, []):
                emit_tail_pair(i, b, ot, j0)
            # first-half output DMA once blocks 0-3 (qh=0) tails are emitted
            if i == 4 * NS + TAIL_DELAY + 5:
                nc.gpsimd.dma_start(
                    out_d.rearrange("(i p) c -> p i c", i=NS)[:, 0:8, :],
                    out_all[:, 0:8 * QDIM].rearrange("p (i c) -> p i c", i=8))
            if i < NU:
                ps_prev = ps
        assert not tails and not ats


_NC = None
LAST_RESULTS = None
LAST_IN_MAPS = None


def kernel(h, wq_w, wq_b, wk_w, wk_b, wv_w, wv_b, **kw):
    global _NC, LAST_RESULTS, LAST_IN_MAPS
    if _NC is None:
        _NC = build_program()

    h = np.asarray(h, np.float32)
    wq_w = np.asarray(wq_w, np.float32)
    wq_b = np.asarray(wq_b, np.float32)
    wk_w = np.asarray(wk_w, np.float32)
    wk_b = np.asarray(wk_b, np.float32)
    wv_w = np.asarray(wv_w, np.float32)
    wv_b = np.asarray(wv_b, np.float32)

    in_maps = []
    for core in range(8):
        b, g = divmod(core, NG)
        # fold the 1/sqrt(dh) score scale into wq/bq
        wq_s = wq_w[g * QDIM:(g + 1) * QDIM, :] * 0.125
        bq_s = wq_b[g * QDIM:(g + 1) * QDIM] * 0.125
        wk_s = wk_w[g * DH:(g + 1) * DH, :]
        wv_s = wv_w[g * DH:(g + 1) * DH, :]
        wkv = np.concatenate([wk_s, wv_s], axis=0)          # [128, 1024]
        bkv = np.concatenate([wk_b[g * DH:(g + 1) * DH],
                              wv_b[g * DH:(g + 1) * DH]])   # [128]
        in_maps.append({
            "hT": np.ascontiguousarray(h[b].T).astype(NPBF),
            "wqT": np.ascontiguousarray(wq_s.T).astype(NPBF),
            "wkvT": np.ascontiguousarray(wkv.T).astype(NPBF),
            "bq": np.ascontiguousarray(bq_s.reshape(QDIM, 1)),
            "bkv": np.ascontiguousarray(bkv.reshape(128, 1)),
        })

    res = run_bass_kernel_spmd(_NC, in_maps, core_ids=list(range(8)))
    LAST_RESULTS = res
    LAST_IN_MAPS = in_maps

    out = np.empty((BS, SEQ, 1024), np.float32)
    for core in range(8):
        b, g = divmod(core, NG)
        out[b, :, g * QDIM:(g + 1) * QDIM] = res.results[core]["out"].astype(
            np.float32)
    return out


def bench_exec_ns(reps=8, iters=4):
    """Per-NEFF-execution time: stream `reps` async dispatches with
    device-resident inputs; subtract a reps=1 launch and divide."""
    import time

    import jax
    from jax.sharding import Mesh, NamedSharding, PartitionSpec
    from jax.experimental.shard_map import shard_map

    from concourse import bass2jax, mybir as _mb

    assert _NC is not None and LAST_IN_MAPS is not None, "call kernel() first"
    nc = _NC
    bass2jax.install_neuronx_cc_hook()
    partition_name = (nc.partition_id_tensor.name
                      if nc.partition_id_tensor else None)

    in_names, out_names, out_avals, zero_outs = [], [], [], []
    for alloc in nc.m.functions[0].allocations:
        if not isinstance(alloc, _mb.MemoryLocationSet):
            continue
        name = alloc.memorylocations[0].name
        if alloc.kind == "ExternalInput":
            if name != partition_name:
                in_names.append(name)
        elif alloc.kind == "ExternalOutput":
            out_names.append(name)
            shape = tuple(alloc.tensor_shape)
            dtype = _mb.dt.np(alloc.dtype)
            out_avals.append(jax.core.ShapedArray(shape, dtype))
            zero_outs.append(np.zeros(shape, dtype))
    n_params = len(in_names)
    all_in_names = in_names + out_names
    if partition_name is not None:
        all_in_names.append(partition_name)

    def _body(*args):
        ins = list(args[:n_params])
        outs = list(args[n_params:])
        pid = ([bass2jax.partition_id_tensor()]
               if partition_name is not None else [])
        outs = list(bass2jax._bass_exec_p.bind(
            *ins, *outs, *pid,
            out_avals=tuple(out_avals),
            in_names=tuple(all_in_names),
            out_names=tuple(out_names),
            lowering_input_output_aliases=(),
            sim_require_finite=True,
            sim_require_nnan=True,
            nc=nc,
        ))
        return tuple(outs)

    devices = jax.devices()[:8]
    mesh = Mesh(np.asarray(devices), ("core",))
    spec = PartitionSpec("core")
    n_outs = len(out_names)
    concat_in = [
        np.concatenate([np.asarray(m[name]) for m in LAST_IN_MAPS], axis=0)
        for name in in_names
    ]
    concat_zeros = [np.zeros((8 * z.shape[0], *z.shape[1:]), z.dtype)
                    for z in zero_outs]
    sh = NamedSharding(mesh, spec)
    dev_args = [jax.device_put(a, sh) for a in concat_in + concat_zeros]

    fn = jax.jit(shard_map(_body, mesh=mesh,
                           in_specs=(spec,) * (n_params + n_outs),
                           out_specs=(spec,) * n_outs, check_rep=False))
    r = fn(*dev_args)  # compile + warm
    jax.block_until_ready(r)

    times = {}
    for n in (1, reps):
        best = float("inf")
        for _ in range(iters):
            t0 = time.perf_counter()
            rs = [fn(*dev_args) for _ in range(n)]
            jax.block_until_ready(rs)
            best = min(best, time.perf_counter() - t0)
        times[n] = best
    per_exec = (times[reps] - times[1]) / (reps - 1)
    return per_exec * 1e9, times
